# revision 35
# baseline (speedup 1.0000x reference)
"""CAB (channel-attention block) Trainium2 kernel, v2.

Zero-copy host I/O: inputs arrive channel-split (core = (batch, ch-half) =
input1.reshape(8, 32, 256*256) shards), outputs leave channel-split
(y concat [8,32,256,256] -> reshape [4,64,256,256] view).

On device (uniform SPMD program, per-core asymmetry only via the
partition-id register g = pid % 2 and cond-gated DMAs):
  1. cast f32 shard -> bf16, AllGather over the batch pair -> full-channel
     bf16 image x1f/x2f in DRAM.
  2. window load: [128 part: x1|x2] resident SBUF tile, 132 padded row
     slots x 258 cols (halo rows via cond-gated DMAs, zero rows/cols via
     memset), row base = g*128 via dynamic AP offset.
  3. pass1: dense-3x3-folded q|k conv as 9 packed bf16 matmuls per
     512-chunk (lhsT [128,128] = blockdiag(Wq9^T, Wk9^T)); norms via
     Square-accum; S += Tq^T @ Tk via PE transposes.
  4. tiny AllReduce of S and norm partials over the pair.
  5. softmax (blockdiag mask) -> fold proj/attn/v-conv into per-tap
     G matrices -> gt (bf16).
  6. pass2: out rows = 9 bf16 matmuls per row from the same window tile,
     write ybuf [64, 128, 256].
  7. AllGather ybuf over the pair; dynamic-offset copy of my 32-channel
     slice -> y [32, 256, 256].
"""
import sys

sys.path.insert(0, "/opt/trn_rl_repo")

import numpy as np

import concourse.bacc as bacc
import concourse.bass as bass
import concourse.tile as tile
from concourse import mybir
from concourse.bass import AP

F32 = mybir.dt.float32
BF16 = mybir.dt.bfloat16

B, C, H, W = 4, 64, 256, 256
HEADS = 8
HD = C // HEADS
EPS = 1e-12

HH = H // 2            # rows per core
R = W + 2              # padded row length (258)
NR = HH + 4            # window slots: zero, halo, 128 data, halo, zero
NBUF = NR * R          # 34056
NOUT = HH * R          # 33024 output positions incl. col pads

TAPS = [(dy, dx) for dy in (-1, 0, 1) for dx in (-1, 0, 1)]
TAP_OFF = [2 * R + dy * R + dx for dy, dx in TAPS]

GROUPS = [[0, 1], [2, 3], [4, 5], [6, 7]]

_CACHE = {}


def _pad_positions(start, length):
    """Contiguous runs of pad columns (global col % R in {0, R-1}) within
    [start, start+length), as (offset_rel, run_len) with runs <= 2."""
    runs = []
    end = start + length
    p = (start // R) * R - 1
    while p < end:
        for q in (p, p + 1):
            lo = max(q, start)
            hi = min(q + 1, end)
            if lo < hi:
                if runs and runs[-1][0] + runs[-1][1] == lo - start:
                    runs[-1] = (runs[-1][0], runs[-1][1] + (hi - lo))
                else:
                    runs.append((lo - start, hi - lo))
        p += R
    return runs


def _dyn(sap: AP, extra) -> AP:
    """AP with a runtime (register) offset added; keeps the static offset
    for tile-framework dependency tracking."""
    return AP(
        tensor=sap.tensor,
        offset=sap.offset + extra,
        ap=sap.ap,
        dep_tracking_offset=sap.offset,
    )


def build_module():
    nc = bacc.Bacc("TRN2", target_bir_lowering=False, debug=False, num_devices=8)

    x1 = nc.declare_dram_parameter("x1", [C // 2, H * W], F32, isOutput=False)
    x2 = nc.declare_dram_parameter("x2", [C // 2, H * W], F32, isOutput=False)
    lqkT = nc.declare_dram_parameter("lqkT", [128, 9 * 128], BF16, isOutput=False)
    wv9 = nc.declare_dram_parameter("wv9", [C, 9 * C], F32, isOutput=False)
    pT = nc.declare_dram_parameter("pT", [C, C], F32, isOutput=False)
    temp64 = nc.declare_dram_parameter("temp64", [C, 1], F32, isOutput=False)
    identf = nc.declare_dram_parameter("identf", [128, 128], F32, isOutput=False)
    identb = nc.declare_dram_parameter("identb", [128, 128], BF16, isOutput=False)
    mask64 = nc.declare_dram_parameter("mask64", [C, C], F32, isOutput=False)
    hmask = nc.declare_dram_parameter("hmask", [128, 2], F32, isOutput=False)
    y = nc.declare_dram_parameter("y", [C // 2, H, W], F32, isOutput=True)
    ydbg = nc.declare_dram_parameter("ydbg", [128, 2 * (C + 1)], F32, isOutput=True)

    with tile.TileContext(nc) as tc:
        _body(tc, nc, x1, x2, lqkT, wv9, pT, temp64, identf, identb, mask64, hmask, y, ydbg)
    nc.compile()
    return nc


def _body(tc, nc, x1, x2, lqkT, wv9, pT, temp64, identf, identb, mask64, hmask, y, ydbg):
    mm = nc.tensor.matmul
    f = F32
    bf = BF16

    wpool = tc.alloc_tile_pool(name="weights", bufs=1)
    dram = tc.alloc_tile_pool(name="dram", bufs=1, space="DRAM")
    accp = tc.alloc_tile_pool(name="ps_acc", bufs=1, space=bass.MemorySpace.PSUM)
    persist = tc.alloc_tile_pool(name="persist", bufs=1)

    w_lqkT = wpool.tile([128, 9 * 128], bf)
    nc.gpsimd.dma_start(w_lqkT[:], lqkT[:])
    w_wv9 = wpool.tile([C, 9 * C], f)
    nc.gpsimd.dma_start(w_wv9[:], wv9[:])
    w_pT = wpool.tile([C, C], f)
    nc.gpsimd.dma_start(w_pT[:], pT[:])
    w_temp = wpool.tile([C, 1], f)
    nc.gpsimd.dma_start(w_temp[:], temp64[:])
    w_idf = wpool.tile([128, 128], f)
    nc.gpsimd.dma_start(w_idf[:], identf[:])
    w_idb = wpool.tile([128, 128], bf)
    nc.gpsimd.dma_start(w_idb[:], identb[:])
    w_mask = wpool.tile([C, C], f)
    nc.gpsimd.dma_start(w_mask[:], mask64[:])
    w_hmask = wpool.tile([128, 2], f)
    nc.gpsimd.dma_start(w_hmask[:], hmask[:])

    # partition-id registers on the engines that issue dynamic DMAs
    g_gp = nc.gpsimd.partition_id() % 2

    # ---------------- stages 0-2: cast -> AllGather -> window, pipelined --
    # Quarter granularity: (half h, quarter q) = 32 rows. Each quarter has
    # its own bf16 staging + gathered DRAM tile so tile-level dependency
    # tracking pipelines cast / collective / window-load / pass1 exactly.
    NQ = 4
    QR = HH // NQ                      # 32 rows per quarter
    QE = QR * W                        # elements per channel-quarter (8192)
    xw = persist.tile([128, NBUF], bf, tag="xw")
    zr = xw[:].rearrange("p (s c) -> p s c", s=NR)
    for s in (0, 1, 130, 131):
        nc.vector.memset(zr[:, s, :], 0.0)
    nc.vector.memset(AP(tensor=zr.tensor, offset=2 * R,
                        ap=[[zr.ap[0][0], 128], [R, 128], [257, 2]]), 0.0)

    # Per-input gathered tensor [2 halves, 64ch, 128 rows, 256]; each
    # quarter AG writes a strided slice of it. Window loads then pick the
    # right half with a dynamic (partition-id register) offset -- no
    # cond-gated DMAs anywhere. Halo slots are always written from a
    # g-dependent source row, then multiplied by a 0/1 host mask.
    # NOTE: cast pools stay alive until the end -- releasing them early
    # lets pass1's pools reuse their SBUF while cast DMAs are still in
    # flight on other queues.
    cin = tc.alloc_tile_pool(name="cast_in", bufs=3)
    cout = tc.alloc_tile_pool(name="cast_out", bufs=3)
    HBQ = C * QR * W                   # elements per (half, quarter) block
    gbig = []
    for ti_idx in range(2):
        gb = dram.tile([2, NQ, C, QR, W], bf, tag=f"gbig{ti_idx}")
        gbig.append(gb)
    # AG order: top-halo sources (h0q3) first, then q0..q2, q3's h1 last.
    # Within each quarter h0 precedes h1; the cc stream completes in
    # order, so a dep on the h1 region covers the h0 region too.
    AG_ORDER = ([(ti, 0, 3) for ti in range(2)]
                + [(ti, h, q) for q in range(3) for ti in range(2)
                   for h in range(2)]
                + [(ti, 1, 3) for ti in range(2)])
    for ti_idx, h, q in AG_ORDER:
        src = (x1, x2)[ti_idx]
        off = (h * NQ + q) * QE
        sv = src[:, off:off + QE].rearrange(
            "c (s k) -> c s k", s=4)            # [32, 4, 2048]
        tin = cin.tile([128, QE // 4], f)
        nc.sync.dma_start(tin[:], sv)
        tbf = cout.tile([128, QE // 4], bf)
        nc.vector.tensor_copy(tbf[:], tin[:])
        stg = dram.tile([C // 2, QE], bf, tag=f"stg{ti_idx}_{h}_{q}")
        nc.scalar.dma_start(
            stg[:].rearrange("c (s k) -> c s k", s=4), tbf[:])
        nc.gpsimd.collective_compute(
            "AllGather", mybir.AluOpType.bypass, replica_groups=GROUPS,
            ins=[stg.opt()], outs=[gbig[ti_idx][h, q].opt()])

    # window loads, interleaved x1/x2 so neither blocks the other on the
    # gpsimd queue; halos (slot 1 / 130) always written then masked.
    for po, gb in ((0, gbig[0]), (64, gbig[1])):
        # top halo: global row 127 = h0q3 row 31 for g==1; junk row 0
        # (masked to zero) for g==0.
        nc.gpsimd.dma_start(zr[po:po + 64, 1:2, 1:257],
                            _dyn(gb[0, 3, :, 0:1, :], g_gp * ((QR - 1) * W)))
    for q in range(NQ):
        for po, gb in ((0, gbig[0]), (64, gbig[1])):
            dst = zr[po:po + 64, 2 + q * QR:2 + (q + 1) * QR, 1:257]
            # static AP = h1 quarter; dynamic (g-1)*NQ*HBQ rebases to h=g.
            nc.gpsimd.dma_start(
                dst, _dyn(gb[1, q], (g_gp - 1) * (NQ * HBQ)))
        if q == 0:
            for po, gb in ((0, gbig[0]), (64, gbig[1])):
                # bottom halo: global row 128 = h1q0 row 0 for g==0;
                # junk h1q0 row 31 (masked) for g==1.
                nc.gpsimd.dma_start(
                    zr[po:po + 64, 130:131, 1:257],
                    _dyn(gb[1, 0, :, 0:1, :], g_gp * ((QR - 1) * W)))
    # zero the junk halo rows on cores at the image boundary
    nc.vector.tensor_scalar(zr[:, 1, :], zr[:, 1, :], w_hmask[:, 0:1],
                            None, op0=mybir.AluOpType.mult)
    nc.vector.tensor_scalar(zr[:, 130, :], zr[:, 130, :], w_hmask[:, 1:2],
                            None, op0=mybir.AluOpType.mult)

    acc_ps = accp.tile([C, C], f)          # S accumulator (q.kT)
    qk2 = persist.tile([128, 1], f, tag="qk2")
    nc.vector.memset(qk2[:], 0.0)

    # ---------------- pass 1: q,k conv -> norms, S -----------------------
    # Software-pipelined so the in-order PE stream never waits on scalar /
    # vector drains: mms of chunk k, then transposes of chunk k-1 (input:
    # qkc copied by scalar during chunk k), then S-matmuls of chunk k-2
    # (input: trsb copied by vector during chunk k-1).
    SPAN = 512
    n_sub_total = NOUT // 128
    sub_idx = 0
    with (
        tc.tile_pool(name="qkc", bufs=3) as qkcp,
        tc.tile_pool(name="trsb", bufs=3) as trsbp,
        tc.tile_pool(name="scratch", bufs=2) as scrp,
        tc.tile_pool(name="ps_conv", bufs=2, space=bass.MemorySpace.PSUM) as pcv,
        tc.tile_pool(name="ps_tr", bufs=2, space=bass.MemorySpace.PSUM) as ptr,
    ):
        chunks = [(p0, min(SPAN, NOUT - p0)) for p0 in range(0, NOUT, SPAN)]
        tr_pend = []    # (qkc, L) awaiting transposes
        s_pend = []     # (trsb, L) awaiting S-matmuls

        def emit_tr():
            qkc, L = tr_pend.pop(0)
            trps = ptr.tile([128, SPAN], bf, tag="trps")
            for j in range(0, L, 128):
                nc.tensor.transpose(trps[:, j:j + 128], qkc[:, j:j + 128],
                                    w_idb[:])
            trsb = trsbp.tile([128, SPAN], bf)
            nc.vector.tensor_copy(trsb[:, 0:L], trps[:, 0:L])
            s_pend.append((trsb, L))

        def emit_s():
            nonlocal sub_idx
            trsb, L = s_pend.pop(0)
            for j in range(0, L, 128):
                mm(acc_ps[:], trsb[:, j:j + C], trsb[:, j + C:j + 128],
                   start=(sub_idx == 0), stop=(sub_idx == n_sub_total - 1))
                sub_idx += 1

        for k, (p0, L) in enumerate(chunks):
            qkps = pcv.tile([128, SPAN], f, tag="qkps")
            for t in range(9):
                o = p0 + TAP_OFF[t]
                # taps-outer: one weight load covers both 512-halves
                for c0 in range(0, L, 512):
                    ce = min(c0 + 512, L)
                    mm(qkps[:, c0:ce], w_lqkT[:, t * 128:(t + 1) * 128],
                       xw[:, o + c0:o + ce], start=(t == 0), stop=(t == 8))
            if len(tr_pend) == 2:
                emit_tr()
            if len(s_pend) == 2:
                emit_s()
            qkc = qkcp.tile([128, SPAN], bf)
            nc.scalar.copy(qkc[:, 0:L], qkps[:, 0:L])
            for off, ln in _pad_positions(p0, L):
                nc.gpsimd.memset(qkc[:, off:off + ln], 0.0)
            sq = scrp.tile([128, SPAN], bf)
            acc_tmp = scrp.tile([128, 1], f, tag="acctmp")
            nc.scalar.activation(
                sq[:, 0:L], qkc[:, 0:L],
                mybir.ActivationFunctionType.Square, accum_out=acc_tmp[:])
            nc.vector.tensor_add(qk2[:], qk2[:], acc_tmp[:])
            tr_pend.append((qkc, L))
        while tr_pend:
            emit_tr()
            if len(s_pend) == 2:
                emit_s()
        while s_pend:
            emit_s()

    # ---------------- collective: S and norms over the batch pair --------
    cc_sb = persist.tile([128, C + 1], f, tag="ccsb")
    nc.vector.memset(cc_sb[:], 0.0)
    nc.scalar.copy(cc_sb[0:C, 0:C], acc_ps[:])
    nc.vector.tensor_copy(cc_sb[:, C:C + 1], qk2[:])
    cc_in = dram.tile([128, C + 1], f, tag="cc_in")
    cc_out = dram.tile([128, C + 1], f, tag="cc_out")
    nc.sync.dma_start(cc_in[:], cc_sb[:])
    nc.gpsimd.collective_compute(
        "AllReduce", mybir.AluOpType.add, replica_groups=GROUPS,
        ins=[cc_in.opt()], outs=[cc_out.opt()])
    sqk = persist.tile([128, C + 1], f, tag="sqk")
    nc.sync.dma_start(sqk[:], cc_out[:])
    nc.sync.dma_start(ydbg[:, 0:C + 1], cc_sb[:])
    nc.sync.dma_start(ydbg[:, C + 1:], sqk[:])

    # ---------------- tiny mid-section: softmax, M^T, G^T ----------------
    gtb = persist.tile([128, 9 * C], bf, tag="gtb")
    with (
        tc.tile_pool(name="mid", bufs=1) as midp,
        tc.tile_pool(name="ps_mid", bufs=1, space=bass.MemorySpace.PSUM) as pmid,
    ):
        nrm = midp.tile([128, 1], f, tag="nrm")
        nc.scalar.sqrt(nrm[:], sqk[:, C:C + 1])
        nc.vector.tensor_scalar_max(nrm[:], nrm[:], EPS)
        rn = midp.tile([128, 1], f, tag="rn")
        nc.vector.reciprocal(rn[:], nrm[:])
        rs = midp.tile([C, 1], f, tag="rs")
        nc.vector.tensor_mul(rs[:], rn[0:C, :], w_temp[:])

        nkT_ps = pmid.tile([1, C], f, tag="nkT")
        nc.tensor.transpose(nkT_ps[:], rn[C:128, :], w_idf[C:128, C:128])
        nkT = midp.tile([1, C], f, tag="nkT_sb")
        nc.scalar.copy(nkT[:], nkT_ps[:])
        ones1 = midp.tile([1, C], f, tag="ones1")
        nc.vector.memset(ones1[:], 1.0)
        nkb_ps = pmid.tile([C, C], f, tag="nkb")
        mm(nkb_ps[:], ones1[:], nkT[:])
        sp = midp.tile([C, C], f, tag="sp")
        nc.vector.tensor_scalar(sp[:], sqk[0:C, 0:C], rs[:], None,
                                op0=mybir.AluOpType.mult)
        nc.vector.tensor_mul(sp[:], sp[:], nkb_ps[:])

        nc.vector.tensor_add(sp[:], sp[:], w_mask[:])
        negm = midp.tile([C, 1], f, tag="negm")
        nc.vector.tensor_reduce(negm[:], sp[:], axis=mybir.AxisListType.X,
                                op=mybir.AluOpType.max, negate=True)
        den = midp.tile([C, 1], f, tag="den")
        ex = midp.tile([C, C], f, tag="ex")
        nc.scalar.activation(ex[:], sp[:], mybir.ActivationFunctionType.Exp,
                             bias=negm[:], scale=1.0, accum_out=den[:])
        rden = midp.tile([C, 1], f, tag="rden")
        nc.vector.reciprocal(rden[:], den[:])
        ablk = midp.tile([C, C], f, tag="ablk")
        nc.vector.tensor_scalar(ablk[:], ex[:], rden[:], None,
                                op0=mybir.AluOpType.mult)

        mt_ps = pmid.tile([C, C], f, tag="mt")
        mm(mt_ps[:], ablk[:], w_pT[:])
        mt = midp.tile([C, C], f, tag="mt_sb")
        nc.scalar.copy(mt[:], mt_ps[:])

        # G^T[dp] = wv9[dp] @ M^T, on partitions 64:128 (pass2 rhs lives there)
        gt_ps = pmid.tile([128, 9 * C], f, tag="gt")
        for dp in range(9):
            s = dp * C
            mm(gt_ps[C:128, s:s + C], w_wv9[:, s:s + C], mt[:],
               tile_position=(0, 64))
        nc.scalar.copy(gtb[C:128, :], gt_ps[C:128, :])

    accp.release()

    # ---------------- pass 2: out = G-conv(x2 window), write ybuf --------
    # per-quarter ybuf tiles so the output AllGathers pipeline behind pass2
    ybufs, yscrs = [], []
    for k in range(NQ):
        ybuf_k = dram.tile([C, QR, W], f, tag=f"ybuf{k}")
        yscr_k = dram.tile([2, C, QR, W], f, tag=f"yscr{k}")
        ybufs.append(ybuf_k)
        yscrs.append(yscr_k)
    yv = y[:].rearrange("c (s r) w -> c s r w", s=2)
    # taps-outer over 8-row groups: one weight load per tap feeds 8
    # matmuls into 8 PSUM banks, amortizing LDWEIGHTS 8x.
    RG = 8
    with (
        tc.tile_pool(name="osb", bufs=2 * RG) as osbp,
        tc.tile_pool(name="ps_p2", bufs=1, space=bass.MemorySpace.PSUM) as pp2,
    ):
        for r0 in range(0, HH, RG):
            o2s = []
            for rr in range(RG):
                o2_t = pp2.tile([C, R], f, tag=f"o2_{rr}")
                o2s.append(o2_t)
            for t in range(9):
                for rr in range(RG):
                    o = (r0 + rr) * R + TAP_OFF[t]
                    mm(o2s[rr][:], gtb[C:128, t * C:(t + 1) * C],
                       xw[C:128, o:o + R], start=(t == 0), stop=(t == 8),
                       tile_position=(64, 0))
            for rr in range(RG):
                r = r0 + rr
                osb = osbp.tile([C, R], f)
                nc.vector.tensor_copy(osb[:], o2s[rr][:])
                nc.sync.dma_start(ybufs[r // QR][:, r % QR, :],
                                  osb[:, 1:W + 1])
            if r % QR == QR - 1:
                k = r // QR
                nc.gpsimd.collective_compute(
                    "AllGather", mybir.AluOpType.bypass,
                    replica_groups=GROUPS,
                    ins=[ybufs[k].opt()], outs=[yscrs[k].opt()])
                # y[c, s*128 + k*QR + r', w] = yscr[s, g*32 + c, r', w]
                for s in range(2):
                    src = yscrs[k][s, 0:C // 2, :, :]
                    dst = yv[:, s, k * QR:(k + 1) * QR, :]
                    nc.gpsimd.dma_start(
                        dst, _dyn(src, g_gp * ((C // 2) * QR * W)))

    for p in (cout, cin, persist, dram, wpool):
        p.release()


# ======================= host side =========================================

def _prep_consts(q_w, q_dw_w, kv_w, kv_dw_w, proj_w, temperature):
    import ml_dtypes
    bf = ml_dtypes.bfloat16

    q_w = np.asarray(q_w, np.float32)[:, :, 0, 0]          # [o, i]
    kv_w = np.asarray(kv_w, np.float32)[:, :, 0, 0]        # [2C, i]
    q_dw = np.asarray(q_dw_w, np.float32)[:, 0]            # [C, 3, 3]
    kv_dw = np.asarray(kv_dw_w, np.float32)[:, 0]          # [2C, 3, 3]
    proj = np.asarray(proj_w, np.float32)[:, :, 0, 0]      # [o, c]
    temp = np.asarray(temperature, np.float32).reshape(HEADS)

    lqkT = np.zeros((128, 9 * 128), np.float32)
    wv9 = np.zeros((C, 9 * C), np.float32)
    for t, (dy, dx) in enumerate(TAPS):
        w9q = q_dw[:, dy + 1, dx + 1][:, None] * q_w       # [o, i]
        w9k = kv_dw[0:C, dy + 1, dx + 1][:, None] * kv_w[0:C]
        lqkT[0:C, t * 128:t * 128 + C] = w9q.T
        lqkT[C:128, t * 128 + C:(t + 1) * 128] = w9k.T
        wv9[:, t * C:(t + 1) * C] = (
            kv_dw[C:2 * C, dy + 1, dx + 1][:, None] * kv_w[C:2 * C]
        )
    pTm = proj.T.copy()
    temp64 = np.repeat(temp, HD).reshape(C, 1).astype(np.float32)
    identf = np.eye(128, dtype=np.float32)
    identb = np.eye(128, dtype=bf)
    mask = np.full((C, C), -1e30, np.float32)
    for h in range(HEADS):
        mask[h * HD:(h + 1) * HD, h * HD:(h + 1) * HD] = 0.0
    return {
        "lqkT": lqkT.astype(bf), "wv9": wv9, "pT": pTm, "temp64": temp64,
        "identf": identf, "identb": identb, "mask64": mask,
    }


def make_in_arrays(input1, input2, q_w, q_dw_w, kv_w, kv_dw_w, proj_w,
                   temperature, n_cores=8):
    """Full concatenated input arrays keyed by param name (big ones are
    zero-copy views of input1/input2)."""
    consts = _prep_consts(q_w, q_dw_w, kv_w, kv_dw_w, proj_w, temperature)
    input1 = np.asarray(input1, np.float32)
    input2 = np.asarray(input2, np.float32)
    arrs = {
        "x1": input1.reshape(n_cores * (C // 2), H * W),
        "x2": input2.reshape(n_cores * (C // 2), H * W),
    }
    for k, v in consts.items():
        arrs[k] = np.tile(v, (n_cores,) + (1,) * (v.ndim - 1))
    # per-core halo masks: [slot1 keep, slot130 keep] = [g, 1-g]
    hm = np.empty((n_cores, 128, 2), np.float32)
    for core in range(n_cores):
        g = core % 2
        hm[core, :, 0] = g
        hm[core, :, 1] = 1 - g
    arrs["hmask"] = hm.reshape(n_cores * 128, 2)
    return arrs


def kernel(input1, input2, q_w, q_dw_w, kv_w, kv_dw_w, proj_w, temperature):
    if "nc" not in _CACHE:
        _CACHE["nc"] = build_module()
    nc = _CACHE["nc"]

    arrs = make_in_arrays(input1, input2, q_w, q_dw_w, kv_w, kv_dw_w,
                          proj_w, temperature)
    out = _get_runner(nc)(arrs)
    return np.asarray(out["y"]).reshape(B, C, H, W)


def _get_runner(nc, n_cores=8):
    """jitted shard_map executable built once and reused across calls."""
    if "runner" in _CACHE:
        return _CACHE["runner"]
    import jax
    from jax.sharding import Mesh, PartitionSpec
    from jax.experimental.shard_map import shard_map
    from concourse import bass2jax as b2j
    from concourse import mybir as _mb

    b2j.install_neuronx_cc_hook()
    partition_name = nc.partition_id_tensor.name if nc.partition_id_tensor else None
    in_names, out_names, out_avals, zero_shapes = [], [], [], []
    for alloc in nc.m.functions[0].allocations:
        if not isinstance(alloc, _mb.MemoryLocationSet):
            continue
        name = alloc.memorylocations[0].name
        if alloc.kind == "ExternalInput":
            if name != partition_name:
                in_names.append(name)
        elif alloc.kind == "ExternalOutput":
            out_names.append(name)
            shape = tuple(alloc.tensor_shape)
            dtype = _mb.dt.np(alloc.dtype)
            out_avals.append(jax.core.ShapedArray(shape, dtype))
            zero_shapes.append((shape, dtype))
    n_params = len(in_names)
    n_outs = len(out_avals)
    all_in_names = list(in_names) + list(out_names)
    if partition_name is not None:
        all_in_names.append(partition_name)
    donate = tuple(range(n_params, n_params + n_outs))

    def _pjrt_body(*args):
        operands = list(args)
        if partition_name is not None:
            operands.append(b2j.partition_id_tensor())
        return tuple(b2j._bass_exec_p.bind(
            *operands, out_avals=tuple(out_avals), in_names=tuple(all_in_names),
            out_names=tuple(out_names), lowering_input_output_aliases=(),
            sim_require_finite=True, sim_require_nnan=True, nc=nc))

    devices = jax.devices()[:n_cores]
    mesh = Mesh(np.asarray(devices), ("core",))
    sharded = jax.jit(
        shard_map(_pjrt_body, mesh=mesh,
                  in_specs=(PartitionSpec("core"),) * (n_params + n_outs),
                  out_specs=(PartitionSpec("core"),) * n_outs, check_rep=False),
        donate_argnums=donate, keep_unused=True)

    def run(arrs):
        concat_in = [arrs[nm] for nm in in_names]
        concat_zeros = [np.zeros((n_cores * s[0], *s[1:]), d)
                        for s, d in zero_shapes]
        out_arrs = sharded(*concat_in, *concat_zeros)
        return {nm: out_arrs[i] for i, nm in enumerate(out_names)}

    _CACHE["runner"] = run
    return run


# revision 36
# speedup vs baseline: 1.0067x; 1.0067x over previous
"""CAB (channel-attention block) Trainium2 kernel, v2.

Zero-copy host I/O: inputs arrive channel-split (core = (batch, ch-half) =
input1.reshape(8, 32, 256*256) shards), outputs leave channel-split
(y concat [8,32,256,256] -> reshape [4,64,256,256] view).

On device (uniform SPMD program, per-core asymmetry only via the
partition-id register g = pid % 2 and cond-gated DMAs):
  1. cast f32 shard -> bf16, AllGather over the batch pair -> full-channel
     bf16 image x1f/x2f in DRAM.
  2. window load: [128 part: x1|x2] resident SBUF tile, 132 padded row
     slots x 258 cols (halo rows via cond-gated DMAs, zero rows/cols via
     memset), row base = g*128 via dynamic AP offset.
  3. pass1: dense-3x3-folded q|k conv as 9 packed bf16 matmuls per
     512-chunk (lhsT [128,128] = blockdiag(Wq9^T, Wk9^T)); norms via
     Square-accum; S += Tq^T @ Tk via PE transposes.
  4. tiny AllReduce of S and norm partials over the pair.
  5. softmax (blockdiag mask) -> fold proj/attn/v-conv into per-tap
     G matrices -> gt (bf16).
  6. pass2: out rows = 9 bf16 matmuls per row from the same window tile,
     write ybuf [64, 128, 256].
  7. AllGather ybuf over the pair; dynamic-offset copy of my 32-channel
     slice -> y [32, 256, 256].
"""
import sys

sys.path.insert(0, "/opt/trn_rl_repo")

import numpy as np

import concourse.bacc as bacc
import concourse.bass as bass
import concourse.tile as tile
from concourse import mybir
from concourse.bass import AP

F32 = mybir.dt.float32
BF16 = mybir.dt.bfloat16

B, C, H, W = 4, 64, 256, 256
HEADS = 8
HD = C // HEADS
EPS = 1e-12

HH = H // 2            # rows per core
R = W + 2              # padded row length (258)
NR = HH + 4            # window slots: zero, halo, 128 data, halo, zero
NBUF = NR * R          # 34056
NOUT = HH * R          # 33024 output positions incl. col pads

TAPS = [(dy, dx) for dy in (-1, 0, 1) for dx in (-1, 0, 1)]
TAP_OFF = [2 * R + dy * R + dx for dy, dx in TAPS]

GROUPS = [[0, 1], [2, 3], [4, 5], [6, 7]]

_CACHE = {}


def _pad_positions(start, length):
    """Contiguous runs of pad columns (global col % R in {0, R-1}) within
    [start, start+length), as (offset_rel, run_len) with runs <= 2."""
    runs = []
    end = start + length
    p = (start // R) * R - 1
    while p < end:
        for q in (p, p + 1):
            lo = max(q, start)
            hi = min(q + 1, end)
            if lo < hi:
                if runs and runs[-1][0] + runs[-1][1] == lo - start:
                    runs[-1] = (runs[-1][0], runs[-1][1] + (hi - lo))
                else:
                    runs.append((lo - start, hi - lo))
        p += R
    return runs


def _dyn(sap: AP, extra) -> AP:
    """AP with a runtime (register) offset added; keeps the static offset
    for tile-framework dependency tracking."""
    return AP(
        tensor=sap.tensor,
        offset=sap.offset + extra,
        ap=sap.ap,
        dep_tracking_offset=sap.offset,
    )


def build_module():
    nc = bacc.Bacc("TRN2", target_bir_lowering=False, debug=False, num_devices=8)

    x1 = nc.declare_dram_parameter("x1", [C // 2, H * W], F32, isOutput=False)
    x2 = nc.declare_dram_parameter("x2", [C // 2, H * W], F32, isOutput=False)
    lqkT = nc.declare_dram_parameter("lqkT", [128, 9 * 128], BF16, isOutput=False)
    wv9 = nc.declare_dram_parameter("wv9", [C, 9 * C], F32, isOutput=False)
    pT = nc.declare_dram_parameter("pT", [C, C], F32, isOutput=False)
    temp64 = nc.declare_dram_parameter("temp64", [C, 1], F32, isOutput=False)
    identf = nc.declare_dram_parameter("identf", [128, 128], F32, isOutput=False)
    identb = nc.declare_dram_parameter("identb", [128, 128], BF16, isOutput=False)
    mask64 = nc.declare_dram_parameter("mask64", [C, C], F32, isOutput=False)
    hmask = nc.declare_dram_parameter("hmask", [128, 2], F32, isOutput=False)
    y = nc.declare_dram_parameter("y", [C // 2, H, W], F32, isOutput=True)
    ydbg = nc.declare_dram_parameter("ydbg", [128, 2 * (C + 1)], F32, isOutput=True)

    with tile.TileContext(nc) as tc:
        _body(tc, nc, x1, x2, lqkT, wv9, pT, temp64, identf, identb, mask64, hmask, y, ydbg)
    nc.compile()
    return nc


def _body(tc, nc, x1, x2, lqkT, wv9, pT, temp64, identf, identb, mask64, hmask, y, ydbg):
    mm = nc.tensor.matmul
    f = F32
    bf = BF16

    wpool = tc.alloc_tile_pool(name="weights", bufs=1)
    dram = tc.alloc_tile_pool(name="dram", bufs=1, space="DRAM")
    accp = tc.alloc_tile_pool(name="ps_acc", bufs=1, space=bass.MemorySpace.PSUM)
    persist = tc.alloc_tile_pool(name="persist", bufs=1)

    w_lqkT = wpool.tile([128, 9 * 128], bf)
    nc.gpsimd.dma_start(w_lqkT[:], lqkT[:])
    w_wv9 = wpool.tile([C, 9 * C], f)
    nc.gpsimd.dma_start(w_wv9[:], wv9[:])
    w_pT = wpool.tile([C, C], f)
    nc.gpsimd.dma_start(w_pT[:], pT[:])
    w_temp = wpool.tile([C, 1], f)
    nc.gpsimd.dma_start(w_temp[:], temp64[:])
    w_idf = wpool.tile([128, 128], f)
    nc.gpsimd.dma_start(w_idf[:], identf[:])
    w_idb = wpool.tile([128, 128], bf)
    nc.gpsimd.dma_start(w_idb[:], identb[:])
    w_mask = wpool.tile([C, C], f)
    nc.gpsimd.dma_start(w_mask[:], mask64[:])
    w_hmask = wpool.tile([128, 2], f)
    nc.gpsimd.dma_start(w_hmask[:], hmask[:])

    # partition-id registers on the engines that issue dynamic DMAs
    g_gp = nc.gpsimd.partition_id() % 2

    # ---------------- stages 0-2: cast -> AllGather -> window, pipelined --
    # Quarter granularity: (half h, quarter q) = 32 rows. Each quarter has
    # its own bf16 staging + gathered DRAM tile so tile-level dependency
    # tracking pipelines cast / collective / window-load / pass1 exactly.
    NQ = 4
    QR = HH // NQ                      # 32 rows per quarter
    QE = QR * W                        # elements per channel-quarter (8192)
    xw = persist.tile([128, NBUF], bf, tag="xw")
    zr = xw[:].rearrange("p (s c) -> p s c", s=NR)
    for s in (0, 1, 130, 131):
        nc.vector.memset(zr[:, s, :], 0.0)
    nc.vector.memset(AP(tensor=zr.tensor, offset=2 * R,
                        ap=[[zr.ap[0][0], 128], [R, 128], [257, 2]]), 0.0)

    # Per-input gathered tensor [2 halves, 64ch, 128 rows, 256]; each
    # quarter AG writes a strided slice of it. Window loads then pick the
    # right half with a dynamic (partition-id register) offset -- no
    # cond-gated DMAs anywhere. Halo slots are always written from a
    # g-dependent source row, then multiplied by a 0/1 host mask.
    # NOTE: cast pools stay alive until the end -- releasing them early
    # lets pass1's pools reuse their SBUF while cast DMAs are still in
    # flight on other queues.
    cin = tc.alloc_tile_pool(name="cast_in", bufs=3)
    cout = tc.alloc_tile_pool(name="cast_out", bufs=3)
    HBQ = C * QR * W                   # elements per (half, quarter) block
    gbig = []
    for ti_idx in range(2):
        gb = dram.tile([2, NQ, C, QR, W], bf, tag=f"gbig{ti_idx}")
        gbig.append(gb)
    # AG order: top-halo sources (h0q3) first, then q0..q2, q3's h1 last.
    # Within each quarter h0 precedes h1; the cc stream completes in
    # order, so a dep on the h1 region covers the h0 region too.
    AG_ORDER = ([(ti, 0, 3) for ti in range(2)]
                + [(ti, h, q) for q in range(3) for ti in range(2)
                   for h in range(2)]
                + [(ti, 1, 3) for ti in range(2)])
    for ti_idx, h, q in AG_ORDER:
        src = (x1, x2)[ti_idx]
        off = (h * NQ + q) * QE
        sv = src[:, off:off + QE].rearrange(
            "c (s k) -> c s k", s=4)            # [32, 4, 2048]
        tin = cin.tile([128, QE // 4], f)
        nc.sync.dma_start(tin[:], sv)
        tbf = cout.tile([128, QE // 4], bf)
        nc.vector.tensor_copy(tbf[:], tin[:])
        stg = dram.tile([C // 2, QE], bf, tag=f"stg{ti_idx}_{h}_{q}")
        nc.scalar.dma_start(
            stg[:].rearrange("c (s k) -> c s k", s=4), tbf[:])
        nc.gpsimd.collective_compute(
            "AllGather", mybir.AluOpType.bypass, replica_groups=GROUPS,
            ins=[stg.opt()], outs=[gbig[ti_idx][h, q].opt()])

    # window loads, interleaved x1/x2 so neither blocks the other on the
    # gpsimd queue; halos (slot 1 / 130) always written then masked.
    for po, gb in ((0, gbig[0]), (64, gbig[1])):
        # top halo: global row 127 = h0q3 row 31 for g==1; junk row 0
        # (masked to zero) for g==0.
        nc.gpsimd.dma_start(zr[po:po + 64, 1:2, 1:257],
                            _dyn(gb[0, 3, :, 0:1, :], g_gp * ((QR - 1) * W)))
    for q in range(NQ):
        for po, gb in ((0, gbig[0]), (64, gbig[1])):
            dst = zr[po:po + 64, 2 + q * QR:2 + (q + 1) * QR, 1:257]
            # static AP = h1 quarter; dynamic (g-1)*NQ*HBQ rebases to h=g.
            nc.gpsimd.dma_start(
                dst, _dyn(gb[1, q], (g_gp - 1) * (NQ * HBQ)))
        if q == 0:
            for po, gb in ((0, gbig[0]), (64, gbig[1])):
                # bottom halo: global row 128 = h1q0 row 0 for g==0;
                # junk h1q0 row 31 (masked) for g==1.
                nc.gpsimd.dma_start(
                    zr[po:po + 64, 130:131, 1:257],
                    _dyn(gb[1, 0, :, 0:1, :], g_gp * ((QR - 1) * W)))
    # zero the junk halo rows on cores at the image boundary
    nc.vector.tensor_scalar(zr[:, 1, :], zr[:, 1, :], w_hmask[:, 0:1],
                            None, op0=mybir.AluOpType.mult)
    nc.vector.tensor_scalar(zr[:, 130, :], zr[:, 130, :], w_hmask[:, 1:2],
                            None, op0=mybir.AluOpType.mult)

    acc_ps = accp.tile([C, C], f)          # S accumulator (q.kT)
    qk2 = persist.tile([128, 1], f, tag="qk2")
    nc.vector.memset(qk2[:], 0.0)

    # ---------------- pass 1: q,k conv -> norms, S -----------------------
    # Software-pipelined so the in-order PE stream never waits on scalar /
    # vector drains: mms of chunk k, then transposes of chunk k-1 (input:
    # qkc copied by scalar during chunk k), then S-matmuls of chunk k-2
    # (input: trsb copied by vector during chunk k-1).
    SPAN = 512
    n_sub_total = NOUT // 128
    sub_idx = 0
    with (
        tc.tile_pool(name="qkc", bufs=3) as qkcp,
        tc.tile_pool(name="trsb", bufs=3) as trsbp,
        tc.tile_pool(name="scratch", bufs=2) as scrp,
        tc.tile_pool(name="ps_conv", bufs=2, space=bass.MemorySpace.PSUM) as pcv,
        tc.tile_pool(name="ps_tr", bufs=2, space=bass.MemorySpace.PSUM) as ptr,
    ):
        chunks = [(p0, min(SPAN, NOUT - p0)) for p0 in range(0, NOUT, SPAN)]
        tr_pend = []    # (qkc, L) awaiting transposes
        s_pend = []     # (trsb, L) awaiting S-matmuls

        def emit_tr():
            qkc, L = tr_pend.pop(0)
            trps = ptr.tile([128, SPAN], bf, tag="trps")
            for j in range(0, L, 128):
                nc.tensor.transpose(trps[:, j:j + 128], qkc[:, j:j + 128],
                                    w_idb[:])
            trsb = trsbp.tile([128, SPAN], bf)
            nc.vector.tensor_copy(trsb[:, 0:L], trps[:, 0:L])
            s_pend.append((trsb, L))

        def emit_s():
            nonlocal sub_idx
            trsb, L = s_pend.pop(0)
            for j in range(0, L, 128):
                mm(acc_ps[:], trsb[:, j:j + C], trsb[:, j + C:j + 128],
                   start=(sub_idx == 0), stop=(sub_idx == n_sub_total - 1))
                sub_idx += 1

        for k, (p0, L) in enumerate(chunks):
            qkps = pcv.tile([128, SPAN], f, tag="qkps")
            for t in range(9):
                o = p0 + TAP_OFF[t]
                # taps-outer: one weight load covers both 512-halves
                for c0 in range(0, L, 512):
                    ce = min(c0 + 512, L)
                    mm(qkps[:, c0:ce], w_lqkT[:, t * 128:(t + 1) * 128],
                       xw[:, o + c0:o + ce], start=(t == 0), stop=(t == 8))
            if len(tr_pend) == 2:
                emit_tr()
            if len(s_pend) == 2:
                emit_s()
            qkc = qkcp.tile([128, SPAN], bf)
            nc.scalar.copy(qkc[:, 0:L], qkps[:, 0:L])
            for off, ln in _pad_positions(p0, L):
                nc.vector.memset(qkc[:, off:off + ln], 0.0)
            sq = scrp.tile([128, SPAN], bf)
            acc_tmp = scrp.tile([128, 1], f, tag="acctmp")
            nc.scalar.activation(
                sq[:, 0:L], qkc[:, 0:L],
                mybir.ActivationFunctionType.Square, accum_out=acc_tmp[:])
            nc.vector.tensor_add(qk2[:], qk2[:], acc_tmp[:])
            tr_pend.append((qkc, L))
        while tr_pend:
            emit_tr()
            if len(s_pend) == 2:
                emit_s()
        while s_pend:
            emit_s()

    # ---------------- collective: S and norms over the batch pair --------
    cc_sb = persist.tile([128, C + 1], f, tag="ccsb")
    nc.vector.memset(cc_sb[:], 0.0)
    nc.scalar.copy(cc_sb[0:C, 0:C], acc_ps[:])
    nc.vector.tensor_copy(cc_sb[:, C:C + 1], qk2[:])
    cc_in = dram.tile([128, C + 1], f, tag="cc_in")
    cc_out = dram.tile([128, C + 1], f, tag="cc_out")
    nc.sync.dma_start(cc_in[:], cc_sb[:])
    nc.gpsimd.collective_compute(
        "AllReduce", mybir.AluOpType.add, replica_groups=GROUPS,
        ins=[cc_in.opt()], outs=[cc_out.opt()])
    sqk = persist.tile([128, C + 1], f, tag="sqk")
    nc.sync.dma_start(sqk[:], cc_out[:])
    nc.sync.dma_start(ydbg[:, 0:C + 1], cc_sb[:])
    nc.sync.dma_start(ydbg[:, C + 1:], sqk[:])

    # ---------------- tiny mid-section: softmax, M^T, G^T ----------------
    gtb = persist.tile([128, 9 * C], bf, tag="gtb")
    with (
        tc.tile_pool(name="mid", bufs=1) as midp,
        tc.tile_pool(name="ps_mid", bufs=1, space=bass.MemorySpace.PSUM) as pmid,
    ):
        nrm = midp.tile([128, 1], f, tag="nrm")
        nc.scalar.sqrt(nrm[:], sqk[:, C:C + 1])
        nc.vector.tensor_scalar_max(nrm[:], nrm[:], EPS)
        rn = midp.tile([128, 1], f, tag="rn")
        nc.vector.reciprocal(rn[:], nrm[:])
        rs = midp.tile([C, 1], f, tag="rs")
        nc.vector.tensor_mul(rs[:], rn[0:C, :], w_temp[:])

        nkT_ps = pmid.tile([1, C], f, tag="nkT")
        nc.tensor.transpose(nkT_ps[:], rn[C:128, :], w_idf[C:128, C:128])
        nkT = midp.tile([1, C], f, tag="nkT_sb")
        nc.scalar.copy(nkT[:], nkT_ps[:])
        ones1 = midp.tile([1, C], f, tag="ones1")
        nc.vector.memset(ones1[:], 1.0)
        nkb_ps = pmid.tile([C, C], f, tag="nkb")
        mm(nkb_ps[:], ones1[:], nkT[:])
        sp = midp.tile([C, C], f, tag="sp")
        nc.vector.tensor_scalar(sp[:], sqk[0:C, 0:C], rs[:], None,
                                op0=mybir.AluOpType.mult)
        nc.vector.tensor_mul(sp[:], sp[:], nkb_ps[:])

        nc.vector.tensor_add(sp[:], sp[:], w_mask[:])
        negm = midp.tile([C, 1], f, tag="negm")
        nc.vector.tensor_reduce(negm[:], sp[:], axis=mybir.AxisListType.X,
                                op=mybir.AluOpType.max, negate=True)
        den = midp.tile([C, 1], f, tag="den")
        ex = midp.tile([C, C], f, tag="ex")
        nc.scalar.activation(ex[:], sp[:], mybir.ActivationFunctionType.Exp,
                             bias=negm[:], scale=1.0, accum_out=den[:])
        rden = midp.tile([C, 1], f, tag="rden")
        nc.vector.reciprocal(rden[:], den[:])
        ablk = midp.tile([C, C], f, tag="ablk")
        nc.vector.tensor_scalar(ablk[:], ex[:], rden[:], None,
                                op0=mybir.AluOpType.mult)

        mt_ps = pmid.tile([C, C], f, tag="mt")
        mm(mt_ps[:], ablk[:], w_pT[:])
        mt = midp.tile([C, C], f, tag="mt_sb")
        nc.scalar.copy(mt[:], mt_ps[:])

        # G^T[dp] = wv9[dp] @ M^T, on partitions 64:128 (pass2 rhs lives there)
        gt_ps = pmid.tile([128, 9 * C], f, tag="gt")
        for dp in range(9):
            s = dp * C
            mm(gt_ps[C:128, s:s + C], w_wv9[:, s:s + C], mt[:],
               tile_position=(0, 64))
        nc.scalar.copy(gtb[C:128, :], gt_ps[C:128, :])

    accp.release()

    # ---------------- pass 2: out = G-conv(x2 window), write ybuf --------
    # per-quarter ybuf tiles so the output AllGathers pipeline behind pass2
    ybufs, yscrs = [], []
    for k in range(NQ):
        ybuf_k = dram.tile([C, QR, W], f, tag=f"ybuf{k}")
        yscr_k = dram.tile([2, C, QR, W], f, tag=f"yscr{k}")
        ybufs.append(ybuf_k)
        yscrs.append(yscr_k)
    yv = y[:].rearrange("c (s r) w -> c s r w", s=2)
    # taps-outer over 8-row groups: one weight load per tap feeds 8
    # matmuls into 8 PSUM banks, amortizing LDWEIGHTS 8x.
    RG = 8
    with (
        tc.tile_pool(name="osb", bufs=2 * RG) as osbp,
        tc.tile_pool(name="ps_p2", bufs=1, space=bass.MemorySpace.PSUM) as pp2,
    ):
        for r0 in range(0, HH, RG):
            o2s = []
            for rr in range(RG):
                o2_t = pp2.tile([C, R], f, tag=f"o2_{rr}")
                o2s.append(o2_t)
            for t in range(9):
                for rr in range(RG):
                    o = (r0 + rr) * R + TAP_OFF[t]
                    mm(o2s[rr][:], gtb[C:128, t * C:(t + 1) * C],
                       xw[C:128, o:o + R], start=(t == 0), stop=(t == 8),
                       tile_position=(64, 0))
            for rr in range(RG):
                r = r0 + rr
                osb = osbp.tile([C, R], f)
                nc.vector.tensor_copy(osb[:], o2s[rr][:])
                nc.sync.dma_start(ybufs[r // QR][:, r % QR, :],
                                  osb[:, 1:W + 1])
            if r % QR == QR - 1:
                k = r // QR
                nc.gpsimd.collective_compute(
                    "AllGather", mybir.AluOpType.bypass,
                    replica_groups=GROUPS,
                    ins=[ybufs[k].opt()], outs=[yscrs[k].opt()])
                # y[c, s*128 + k*QR + r', w] = yscr[s, g*32 + c, r', w]
                for s in range(2):
                    src = yscrs[k][s, 0:C // 2, :, :]
                    dst = yv[:, s, k * QR:(k + 1) * QR, :]
                    nc.gpsimd.dma_start(
                        dst, _dyn(src, g_gp * ((C // 2) * QR * W)))

    for p in (cout, cin, persist, dram, wpool):
        p.release()


# ======================= host side =========================================

def _prep_consts(q_w, q_dw_w, kv_w, kv_dw_w, proj_w, temperature):
    import ml_dtypes
    bf = ml_dtypes.bfloat16

    q_w = np.asarray(q_w, np.float32)[:, :, 0, 0]          # [o, i]
    kv_w = np.asarray(kv_w, np.float32)[:, :, 0, 0]        # [2C, i]
    q_dw = np.asarray(q_dw_w, np.float32)[:, 0]            # [C, 3, 3]
    kv_dw = np.asarray(kv_dw_w, np.float32)[:, 0]          # [2C, 3, 3]
    proj = np.asarray(proj_w, np.float32)[:, :, 0, 0]      # [o, c]
    temp = np.asarray(temperature, np.float32).reshape(HEADS)

    lqkT = np.zeros((128, 9 * 128), np.float32)
    wv9 = np.zeros((C, 9 * C), np.float32)
    for t, (dy, dx) in enumerate(TAPS):
        w9q = q_dw[:, dy + 1, dx + 1][:, None] * q_w       # [o, i]
        w9k = kv_dw[0:C, dy + 1, dx + 1][:, None] * kv_w[0:C]
        lqkT[0:C, t * 128:t * 128 + C] = w9q.T
        lqkT[C:128, t * 128 + C:(t + 1) * 128] = w9k.T
        wv9[:, t * C:(t + 1) * C] = (
            kv_dw[C:2 * C, dy + 1, dx + 1][:, None] * kv_w[C:2 * C]
        )
    pTm = proj.T.copy()
    temp64 = np.repeat(temp, HD).reshape(C, 1).astype(np.float32)
    identf = np.eye(128, dtype=np.float32)
    identb = np.eye(128, dtype=bf)
    mask = np.full((C, C), -1e30, np.float32)
    for h in range(HEADS):
        mask[h * HD:(h + 1) * HD, h * HD:(h + 1) * HD] = 0.0
    return {
        "lqkT": lqkT.astype(bf), "wv9": wv9, "pT": pTm, "temp64": temp64,
        "identf": identf, "identb": identb, "mask64": mask,
    }


def make_in_arrays(input1, input2, q_w, q_dw_w, kv_w, kv_dw_w, proj_w,
                   temperature, n_cores=8):
    """Full concatenated input arrays keyed by param name (big ones are
    zero-copy views of input1/input2)."""
    consts = _prep_consts(q_w, q_dw_w, kv_w, kv_dw_w, proj_w, temperature)
    input1 = np.asarray(input1, np.float32)
    input2 = np.asarray(input2, np.float32)
    arrs = {
        "x1": input1.reshape(n_cores * (C // 2), H * W),
        "x2": input2.reshape(n_cores * (C // 2), H * W),
    }
    for k, v in consts.items():
        arrs[k] = np.tile(v, (n_cores,) + (1,) * (v.ndim - 1))
    # per-core halo masks: [slot1 keep, slot130 keep] = [g, 1-g]
    hm = np.empty((n_cores, 128, 2), np.float32)
    for core in range(n_cores):
        g = core % 2
        hm[core, :, 0] = g
        hm[core, :, 1] = 1 - g
    arrs["hmask"] = hm.reshape(n_cores * 128, 2)
    return arrs


def kernel(input1, input2, q_w, q_dw_w, kv_w, kv_dw_w, proj_w, temperature):
    if "nc" not in _CACHE:
        _CACHE["nc"] = build_module()
    nc = _CACHE["nc"]

    arrs = make_in_arrays(input1, input2, q_w, q_dw_w, kv_w, kv_dw_w,
                          proj_w, temperature)
    out = _get_runner(nc)(arrs)
    return np.asarray(out["y"]).reshape(B, C, H, W)


def _get_runner(nc, n_cores=8):
    """jitted shard_map executable built once and reused across calls."""
    if "runner" in _CACHE:
        return _CACHE["runner"]
    import jax
    from jax.sharding import Mesh, PartitionSpec
    from jax.experimental.shard_map import shard_map
    from concourse import bass2jax as b2j
    from concourse import mybir as _mb

    b2j.install_neuronx_cc_hook()
    partition_name = nc.partition_id_tensor.name if nc.partition_id_tensor else None
    in_names, out_names, out_avals, zero_shapes = [], [], [], []
    for alloc in nc.m.functions[0].allocations:
        if not isinstance(alloc, _mb.MemoryLocationSet):
            continue
        name = alloc.memorylocations[0].name
        if alloc.kind == "ExternalInput":
            if name != partition_name:
                in_names.append(name)
        elif alloc.kind == "ExternalOutput":
            out_names.append(name)
            shape = tuple(alloc.tensor_shape)
            dtype = _mb.dt.np(alloc.dtype)
            out_avals.append(jax.core.ShapedArray(shape, dtype))
            zero_shapes.append((shape, dtype))
    n_params = len(in_names)
    n_outs = len(out_avals)
    all_in_names = list(in_names) + list(out_names)
    if partition_name is not None:
        all_in_names.append(partition_name)
    donate = tuple(range(n_params, n_params + n_outs))

    def _pjrt_body(*args):
        operands = list(args)
        if partition_name is not None:
            operands.append(b2j.partition_id_tensor())
        return tuple(b2j._bass_exec_p.bind(
            *operands, out_avals=tuple(out_avals), in_names=tuple(all_in_names),
            out_names=tuple(out_names), lowering_input_output_aliases=(),
            sim_require_finite=True, sim_require_nnan=True, nc=nc))

    devices = jax.devices()[:n_cores]
    mesh = Mesh(np.asarray(devices), ("core",))
    sharded = jax.jit(
        shard_map(_pjrt_body, mesh=mesh,
                  in_specs=(PartitionSpec("core"),) * (n_params + n_outs),
                  out_specs=(PartitionSpec("core"),) * n_outs, check_rep=False),
        donate_argnums=donate, keep_unused=True)

    def run(arrs):
        concat_in = [arrs[nm] for nm in in_names]
        concat_zeros = [np.zeros((n_cores * s[0], *s[1:]), d)
                        for s, d in zero_shapes]
        out_arrs = sharded(*concat_in, *concat_zeros)
        return {nm: out_arrs[i] for i, nm in enumerate(out_names)}

    _CACHE["runner"] = run
    return run
